# revision 24
# baseline (speedup 1.0000x reference)
"""Trainium2 Bass kernel for the MDN module, data-parallel over 8 NeuronCores.

Math per batch row b (reference.py):
    h  = tanh(x @ W1 + b1)                      [B, H]
    f  = h @ W2 + b2 ; mu, logvar = split(f)    [B, N] each
    Vx  = ||x  @ Wv||^2 + 1e-3                  [B, 1]
    Vmu = ||mu @ Wv||^2 + 1e-3                  [B, 1]
    scale = min(BETA*Vx, Vmu) / Vmu
    mu_s  = mu * scale
    fx    = mu_s + exp(0.5*logvar) * eps
    logp_y = 0.5*(B*N*log(2pi) + sum(logvar + (y-mu_s)^2 * exp(-logvar)))

Sharding: pure data parallel, batch split 8 ways; weights replicated.
The scalar logp_y reduction is finished on the host (each core returns
per-partition partial sums), so no collectives are needed.

On-chip layout per core (BC = 1024 rows):
  xT   [N=256 part-chunks, BC]   via PE transpose of x tiles
  hT   [H=2048 part-chunks, BC]  produced by MM1 (W1 chunk stationary)
  f    [128 b, 512]              MM2 (hT chunk stationary, W2 moving)
  u    [128 b, 256]              x@Wv  (xT chunk stationary, Wv moving)
  umu  [128 b, 256]              mu@Wv (muT chunk stationary, Wv moving)
Matmuls run as float32r (full PE rate for moving dim >= 256).
"""

import numpy as np
from contextlib import ExitStack

import concourse.bass as bass
import concourse.bacc as bacc
import concourse.mybir as mybir
import concourse.tile as tile
from concourse.bass_utils import run_bass_kernel_spmd
from concourse.masks import make_identity

B, N, H = 8192, 256, 2048
NCORES = 8
BC = B // NCORES          # 1024 rows per core
P = 128
NBT = BC // P             # 8 batch tiles per core
NH = H // P               # 16 h chunks
NN = N // P               # 2 n chunks
NF = 512                  # fp32 moving-operand max / one psum bank
BETA = 0.99
LOG2PI = float(np.log(2.0 * np.pi))

F32 = mybir.dt.float32
F32R = mybir.dt.float32r
AFT = mybir.ActivationFunctionType
ALU = mybir.AluOpType
AX = mybir.AxisListType

import os
# fp32r streams the PE at 1 cycle/row (vs 4 for fp32): ~4x matmul throughput.
# HW-measured accuracy: fx maxerr 1.5e-4, logp_y 6e-7. Set MDN_F32R=0 for
# full-fp32 matmuls (fx maxerr 7e-6) at ~2.2x the kernel time.
USE_F32R = os.environ.get("MDN_F32R", "1") == "1"
MMDT = F32R if USE_F32R else F32


def _mm(ap):
    return ap


def _body(ctx: ExitStack, tc, x_d, y_d, e_d, w1_d, b1_d, w2_d, b2_d, wv_d,
          fx_d, acc_d):
    nc = tc.nc

    wpool = ctx.enter_context(tc.tile_pool(name="wpool", bufs=1))
    hpool = ctx.enter_context(tc.tile_pool(name="hpool", bufs=1))
    work = ctx.enter_context(tc.tile_pool(name="work", bufs=3))
    pers = ctx.enter_context(tc.tile_pool(name="pers", bufs=1))
    ph_pool = ctx.enter_context(tc.tile_pool(name="ph", bufs=2, space="PSUM"))
    pf_pool = ctx.enter_context(tc.tile_pool(name="pf", bufs=2, space="PSUM"))
    ps_pool = ctx.enter_context(tc.tile_pool(name="ps", bufs=2, space="PSUM"))
    pt_pool = ctx.enter_context(tc.tile_pool(name="pt", bufs=2, space="PSUM"))

    stage = ctx.enter_context(tc.tile_pool(name="stage", bufs=2))

    # ---- constants / weights ----------------------------------------------
    # fp32r matmul operands must be produced by a compute op (walrus BIR
    # verifier: "not rounded to FP32r"), so DMA'd weights pass through an
    # fp32 staging tile and an ACT copy that rounds into the fp32r tile.
    ident = pers.tile([P, P], F32, tag="ident")
    make_identity(nc, ident)
    acc_t = pers.tile([P, NBT], F32, tag="acc")
    cb1 = pers.tile([P, 1], F32, tag="cb1")
    nc.vector.memset(cb1[:], BETA * 1e-3)
    cb2 = pers.tile([P, 1], F32, tag="cb2")
    nc.vector.memset(cb2[:], 1e-3)

    b1_t = pers.tile([P, NH], F32, tag="b1t")
    nc.sync.dma_start(b1_t[:], b1_d[:])
    # b2 arrives host-pre-broadcast as [128, 2N]; added to f with one DVE op.
    b2b_t = pers.tile([P, 2 * N], F32, tag="b2b")
    nc.sync.dma_start(b2b_t[:], b2_d[:])

    def load_rounded(pool, tag, shape, src_ap):
        s = stage.tile(shape, F32, tag="stage", name=f"s_{tag}")
        nc.sync.dma_start(s[:], src_ap)
        t = pool.tile(shape, MMDT, tag=tag, name=tag)
        nc.scalar.copy(t[:], s[:])
        return t

    wv_t = [load_rounded(pers, f"wv{i}", [P, N], wv_d[i * P:(i + 1) * P, :])
            for i in range(NN)]
    w1_t = [load_rounded(wpool, f"w1_{i}", [P, H], w1_d[i * P:(i + 1) * P, :])
            for i in range(NN)]
    w2_t = [load_rounded(wpool, f"w2_{c}", [P, 2 * N],
                         w2_d[c * P:(c + 1) * P, :])
            for c in range(NH)]

    # ---- load x and transpose to xT [n, b] --------------------------------
    x_t = []
    for i in range(NN):
        t = hpool.tile([P, BC], MMDT, tag=f"xt{i}", name=f"xt{i}")
        x_t.append(t)
    for g in range(NBT // 4):          # groups of 4 b-tiles -> [128, 512] psum
        xbs = []
        for k in range(4):
            bt = g * 4 + k
            xb = work.tile([P, N], F32, tag="xb", name="xb", bufs=8)
            nc.sync.dma_start(xb[:], x_d[bt * P:(bt + 1) * P, :])
            xbs.append(xb)
        for i in range(NN):
            ptile = pt_pool.tile([P, NF], F32, tag="pt", name="ptx")
            for k in range(4):
                nc.tensor.transpose(ptile[:, k * P:(k + 1) * P],
                                    xbs[k][:, i * P:(i + 1) * P], ident[:])
            nc.scalar.copy(x_t[i][:, g * NF:(g + 1) * NF], ptile[:])

    # ---- MM1: hT = tanh(W1^T-chunk @ xT + b1) -----------------------------
    h_t = []
    for c in range(NH):
        t = hpool.tile([P, BC], MMDT, tag=f"ht{c}", name=f"ht{c}")
        h_t.append(t)
    for bf in range(BC // NF):
        bs = slice(bf * NF, (bf + 1) * NF)
        for c in range(NH):
            phh = ph_pool.tile([P, NF], F32, tag="ph", name="phh")
            for i in range(NN):
                nc.tensor.matmul(phh[:], _mm(w1_t[i][:, c * P:(c + 1) * P]),
                                 _mm(x_t[i][:, bs]),
                                 start=(i == 0), stop=(i == NN - 1))
            nc.scalar.activation(h_t[c][:, bs], phh[:], AFT.Tanh,
                                 bias=b1_t[:, c:c + 1], scale=1.0)

    # ---- per-b-tile pipeline ---------------------------------------------
    state = {}

    def part1(bt):
        bs = slice(bt * P, (bt + 1) * P)
        st = {}
        # f = h @ W2  (+ b2 added below on DVE)
        pff = pf_pool.tile([P, 2 * N], F32, tag="pf", name="pff")
        for c in range(NH):
            nc.tensor.matmul(pff[:], _mm(h_t[c][:, bs]), _mm(w2_t[c][:]),
                             start=(c == 0), stop=(c == NH - 1))
        # u = x @ Wv
        pu = ps_pool.tile([P, N], F32, tag="ps", name="pu")
        for i in range(NN):
            nc.tensor.matmul(pu[:], _mm(x_t[i][:, bs]), _mm(wv_t[i][:]),
                             start=(i == 0), stop=(i == NN - 1))
        f_sb = work.tile([P, 2 * N], F32, tag="fsb", name="f_sb")
        nc.vector.tensor_add(f_sb[:], pff[:], b2b_t[:])
        mu_sb = f_sb[:, :N]
        sd = work.tile([P, N], F32, tag="sd", name="sd")
        nc.scalar.activation(sd[:], f_sb[:, N:], AFT.Exp, scale=0.5)
        rvar = work.tile([P, N], F32, tag="rvar", name="rvar")
        nc.scalar.activation(rvar[:], f_sb[:, N:], AFT.Exp, scale=-1.0)
        lvs = work.tile([P, 1], F32, tag="lvs", name="lvs")
        nc.vector.tensor_reduce(lvs[:], f_sb[:, N:], axis=AX.X, op=ALU.add)
        # Vx: ACT square then DVE row-sum
        scr = work.tile([P, N], F32, tag="scr", name="scr")
        nc.scalar.activation(scr[:], pu[:], AFT.Square)
        vxs = work.tile([P, 1], F32, tag="vxs", name="vxs")
        nc.vector.tensor_reduce(vxs[:], scr[:], axis=AX.X, op=ALU.add)
        bvx = work.tile([P, 1], F32, tag="bvx", name="bvx")
        nc.scalar.activation(bvx[:], vxs[:], AFT.Identity,
                             bias=cb1[:], scale=BETA)
        st.update(mu_sb=mu_sb, sd=sd, rvar=rvar, lvs=lvs, bvx=bvx)
        return st

    def part2(bt, st):
        bs = slice(bt * P, (bt + 1) * P)
        # muT via PE transpose, then umu = mu @ Wv
        pmt = pt_pool.tile([P, N], F32, tag="pt", name="pmt")
        for i in range(NN):
            nc.tensor.transpose(pmt[:, i * P:(i + 1) * P],
                                st["mu_sb"][:, i * P:(i + 1) * P], ident[:])
        muT = work.tile([P, N], MMDT, tag="muT", name="muT")
        nc.scalar.copy(muT[:], pmt[:])
        pumu = ps_pool.tile([P, N], F32, tag="ps", name="pumu")
        for i in range(NN):
            nc.tensor.matmul(pumu[:], _mm(muT[:, i * P:(i + 1) * P]),
                             _mm(wv_t[i][:]),
                             start=(i == 0), stop=(i == NN - 1))
        scr2 = work.tile([P, N], F32, tag="scr", name="scr2")
        nc.scalar.activation(scr2[:], pumu[:], AFT.Square)
        vmus = work.tile([P, 1], F32, tag="vmus", name="vmus")
        nc.vector.tensor_reduce(vmus[:], scr2[:], axis=AX.X, op=ALU.add)
        vmu = work.tile([P, 1], F32, tag="vmu", name="vmu")
        nc.scalar.activation(vmu[:], vmus[:], AFT.Identity, bias=cb2[:],
                             scale=1.0)
        rv = work.tile([P, 1], F32, tag="rv", name="rv")
        nc.vector.reciprocal(rv[:], vmu[:])
        rr = work.tile([P, 1], F32, tag="rr", name="rr")
        nc.vector.tensor_mul(rr[:], st["bvx"][:], rv[:])
        scl = work.tile([P, 1], F32, tag="scl", name="scl")
        nc.vector.tensor_scalar_min(scl[:], rr[:], 1.0)
        ms = work.tile([P, N], F32, tag="ms", name="ms")
        nc.vector.tensor_scalar_mul(ms[:], st["mu_sb"][:], scl[:])
        ep = work.tile([P, N], F32, tag="ep", name="ep")
        nc.sync.dma_start(ep[:], e_d[bt * P:(bt + 1) * P, :])
        yt = work.tile([P, N], F32, tag="yt", name="yt")
        nc.sync.dma_start(yt[:], y_d[bt * P:(bt + 1) * P, :])
        tt = work.tile([P, N], F32, tag="tt", name="tt")
        nc.vector.tensor_mul(tt[:], st["sd"][:], ep[:])
        fxt = work.tile([P, N], F32, tag="fxt", name="fxt")
        nc.vector.tensor_add(fxt[:], ms[:], tt[:])
        nc.sync.dma_start(fx_d[bt * P:(bt + 1) * P, :], fxt[:])
        dd = work.tile([P, N], F32, tag="dd", name="dd")
        nc.vector.tensor_sub(dd[:], yt[:], ms[:])
        d2 = work.tile([P, N], F32, tag="d2", name="d2")
        nc.scalar.square(d2[:], dd[:])
        q2 = work.tile([P, N], F32, tag="scr", name="q2")
        nc.vector.tensor_mul(q2[:], d2[:], st["rvar"][:])
        qs = work.tile([P, 1], F32, tag="qs", name="qs")
        nc.vector.tensor_reduce(qs[:], q2[:], axis=AX.X, op=ALU.add)
        nc.vector.tensor_add(acc_t[:, bt:bt + 1], qs[:], st["lvs"][:])

    for bt in range(NBT):
        st = part1(bt)
        if bt > 0:
            part2(bt - 1, state[bt - 1])
        state[bt] = st
    part2(NBT - 1, state[NBT - 1])

    nc.sync.dma_start(acc_d[:], acc_t[:])


def build_nc():
    # num_devices=1: each core runs an identical independent program; the
    # host gathers outputs. num_devices>1 would emit an all-core AllReduce
    # barrier at kernel tail, which hangs this runtime (no collective setup).
    nc = bacc.Bacc("TRN2", target_bir_lowering=False, debug=False,
                   enable_asserts=False, num_devices=1)
    x_d = nc.dram_tensor("x", [BC, N], F32, kind="ExternalInput").ap()
    y_d = nc.dram_tensor("y", [BC, N], F32, kind="ExternalInput").ap()
    e_d = nc.dram_tensor("eps", [BC, N], F32, kind="ExternalInput").ap()
    w1_d = nc.dram_tensor("W1", [N, H], F32, kind="ExternalInput").ap()
    b1_d = nc.dram_tensor("b1t", [P, NH], F32, kind="ExternalInput").ap()
    w2_d = nc.dram_tensor("W2", [H, 2 * N], F32, kind="ExternalInput").ap()
    b2_d = nc.dram_tensor("b2", [P, 2 * N], F32, kind="ExternalInput").ap()
    wv_d = nc.dram_tensor("Wv", [N, N], F32, kind="ExternalInput").ap()
    fx_d = nc.dram_tensor("fx", [BC, N], F32, kind="ExternalOutput").ap()
    acc_d = nc.dram_tensor("acc", [P, NBT], F32, kind="ExternalOutput").ap()

    with tile.TileContext(nc) as tc:
        with ExitStack() as ctx:
            _body(ctx, tc, x_d, y_d, e_d, w1_d, b1_d, w2_d, b2_d, wv_d,
                  fx_d, acc_d)
    nc.compile()
    return nc


_NC_CACHE = None
LAST_RESULTS = None


def _get_nc():
    global _NC_CACHE
    if _NC_CACHE is None:
        _NC_CACHE = build_nc()
    return _NC_CACHE


def _shard_inputs(x, y, eps, W1, b1, W2, b2, Wv):
    f = np.float32
    x = np.ascontiguousarray(x, dtype=f)
    y2 = np.ascontiguousarray(np.asarray(y, dtype=f).reshape(B, N))
    e2 = np.ascontiguousarray(np.asarray(eps, dtype=f).reshape(B, N))
    W1 = np.ascontiguousarray(W1, dtype=f)
    b1t = np.ascontiguousarray(np.asarray(b1, dtype=f).reshape(NH, P).T)
    W2 = np.ascontiguousarray(W2, dtype=f)
    b2r = np.ascontiguousarray(
        np.broadcast_to(np.asarray(b2, dtype=f).reshape(1, 2 * N), (P, 2 * N)))
    Wv = np.ascontiguousarray(Wv, dtype=f)
    maps = []
    for c in range(NCORES):
        sl = slice(c * BC, (c + 1) * BC)
        maps.append({
            "x": np.ascontiguousarray(x[sl]),
            "y": np.ascontiguousarray(y2[sl]),
            "eps": np.ascontiguousarray(e2[sl]),
            "W1": W1, "b1t": b1t, "W2": W2, "b2": b2r, "Wv": Wv,
        })
    return maps


def kernel(x, y, eps, W1, b1, W2, b2, Wv, _trace=False):
    global LAST_RESULTS
    nc = _get_nc()
    in_maps = _shard_inputs(x, y, eps, W1, b1, W2, b2, Wv)
    res = run_bass_kernel_spmd(nc, in_maps, list(range(NCORES)), trace=_trace)
    LAST_RESULTS = res
    fx = np.concatenate([res.results[c]["fx"] for c in range(NCORES)],
                        axis=0).reshape(B, 1, N)
    tot = 0.0
    for c in range(NCORES):
        tot += float(res.results[c]["acc"].astype(np.float64).sum())
    logp_y = np.float32(0.5 * (B * N * LOG2PI + tot))
    return fx, logp_y
